# revision 14
# baseline (speedup 1.0000x reference)
"""Bass/Trainium2 kernel for nn_Attention_61194694034287.

Computes, for q,k,v [2,8,2048,64] f32 and mask [2,2048,2048] i32:
    s   = (q @ k^T) / sqrt(8)            # scale by sqrt(n_heads) quirk
    s   = where(mask==0, -1e11, s)
    att = softmax(s, axis=1)             # softmax over the HEAD axis quirk
    z   = att @ v
returns (z, att) like the reference.

Strategy (8 NeuronCores): core c handles batch c//4, query rows
[(c%4)*512, (c%4)*512+512).  All 8 heads live on every core, so the
head-axis softmax is core-local (no collective needed).

Device-side layout is the *transposed* score layout sT[k, q]: the k->q
orientation makes the att @ v contraction a natural PE matmul (k on
partitions for both att^T and v), avoiding any on-device transpose of the
64M-element score tensor.  The host permutes the score slabs back during
unshard (a cheap numpy transpose).

Head-softmax without max-subtraction: score range is ~±17 after scaling,
safely inside fp32 exp.  Masked positions must come out exactly uniform
(1/8) across heads -- reference adds -1e11 which swamps the scores -- so
we set u=exp(s)=1 at masked positions via copy_predicated and the head-sum
is then exactly 8 there.

Head sum runs on the PE as 8 accumulating identity matmuls; normalize
(u * 1/U) on the vector engine; exp on the scalar engine; QK^T and AV on
the PE in float32r (full-rate fp32 path).
"""

import math
import sys
import types

import numpy as np

for _p in ("/opt/trn_rl_repo",):
    if _p not in sys.path:
        sys.path.append(_p)

import concourse.bass as bass
import concourse.tile as tile
from concourse import mybir
from concourse.masks import make_identity
from concourse.vector_clock import ScopedClock

B, H, S, D = 2, 8, 2048, 64
NCORES = 8
SQ = S * B // NCORES          # 512 query rows per core
KB = S // 128                 # 16 k-blocks of 128
ALPHA = 1.0 / math.sqrt(H)    # scale by sqrt(n_heads) (reference quirk)

F32 = mybir.dt.float32
F32R = mybir.dt.float32r
BF16 = mybir.dt.bfloat16
U8 = mybir.dt.uint8

# ---------------------------------------------------------------------------
# Tile's end-of-kernel drain carries one sem-wait per outstanding semaphore;
# the walrus build here rejects >1 sync wait on the SP drain.  Split the
# extras onto single-wait instructions (same engine, before the barrier).
# ---------------------------------------------------------------------------
def _patched_drain_and_barrier(self, tick_clock, wait_clock):
    nc = self.nc
    drain_inst = nc.sync.drain()
    raw = drain_inst.ins
    wait_clock.add_sem_waits(raw, ScopedClock({None: tick_clock.global_clock}))
    si = raw.sync_info
    waits = list(si.on_wait)
    if len(waits) > 1:
        sems_by_name = {h.name: h for h in self.sems.allocated().values()}
        si.on_wait = waits[:1]
        raw.sync_info = si
        for w in waits[1:]:
            nc.sync.wait_ge(sems_by_name[w.ant_name], w.wait_value)
    nc.all_engine_barrier()
    popped = nc._tile_sem_poison_stack.pop()
    assert popped is self._sem_poison
    nc.clear_and_free_semaphores(list(self.sems.allocated().values()))
    nc.all_engine_barrier()


tile.TileContext._drain_and_barrier = _patched_drain_and_barrier

_MAX_WAITS = 1


def _split_excess_waits(nc):
    """walrus rejects instructions with more than ~1 sync wait; hoist the
    extras onto same-engine NOPs inserted right before the instruction."""
    counter = [0]
    for fn in nc.m.functions:
        for bb in fn.blocks:
            new_insts = []
            for inst in bb.instructions:
                si = inst.sync_info
                if si is not None and len(si.on_wait) > _MAX_WAITS:
                    waits = list(si.on_wait)
                    SI = type(si)
                    for w in waits[:-_MAX_WAITS]:
                        counter[0] += 1
                        nop = mybir.InstNoOp(
                            name=f"wsplit-{counter[0]}", ins=[], outs=[])
                        nop.engine = inst.engine
                        nop.sync_info = SI(on_wait=[w], on_update=[])
                        nc.register_instruction(nop, overwrite=True)
                        new_insts.append(nop)
                    si.on_wait = waits[-_MAX_WAITS:]
                    inst.sync_info = si
                new_insts.append(inst)
            bb.instructions = new_insts


def _bcast(ap2d, n, axis=1):
    """[128, F] AP -> [128, n, F] AP with a step-0 (broadcast) middle dim."""
    return bass.AP(
        tensor=ap2d.tensor,
        offset=ap2d.offset,
        ap=[ap2d.ap[0], [0, n], ap2d.ap[1]],
    )


def _build_program():
    nc = bass.Bass("TRN2", target_bir_lowering=False, debug=False,
                   num_devices=NCORES)

    qT_in = nc.dram_tensor("qT", [128, H // 2, SQ], F32R, kind="ExternalInput")
    kT_in = nc.dram_tensor("kT", [KB, 128, H // 2, 128], F32R, kind="ExternalInput")
    v_in = nc.dram_tensor("v", [128, H, KB, D], BF16, kind="ExternalInput")
    mf_in = nc.dram_tensor("mf", [128, KB, SQ], BF16, kind="ExternalInput")
    mib_in = nc.dram_tensor("mib", [128, KB, SQ], BF16, kind="ExternalInput")
    att_out = nc.dram_tensor("att", [KB, 128, H, SQ], BF16, kind="ExternalOutput")
    z_out = nc.dram_tensor("z", [H // 2, 128, SQ], F32, kind="ExternalOutput")

    with tile.TileContext(nc) as tc:
        with (
            tc.tile_pool(name="singles", bufs=1) as singles,
            tc.tile_pool(name="sb_in", bufs=1) as sb_in,
            tc.tile_pool(name="u", bufs=3) as upool,
            tc.tile_pool(name="kt", bufs=3) as ktpool,
            tc.tile_pool(name="att", bufs=3) as apool,
            tc.tile_pool(name="r", bufs=3) as rpool,
            tc.tile_pool(name="zout", bufs=2) as zpool,
            tc.tile_pool(name="spsum", bufs=3, space="PSUM") as spsum,
            tc.tile_pool(name="upsum", bufs=1, space="PSUM") as upsum,
            tc.tile_pool(name="zpsum", bufs=1, space="PSUM") as zpsum,
        ):
            qT_s = sb_in.tile([128, H // 2, SQ], F32R)
            nc.sync.dma_start(qT_s[:], qT_in[:])
            v_s = sb_in.tile([128, H, KB, D], BF16)
            nc.sync.dma_start(v_s[:], v_in[:])
            mf_s = sb_in.tile([128, KB, SQ], BF16)
            nc.sync.dma_start(mf_s[:], mf_in[:])
            mib_s = sb_in.tile([128, KB, SQ], BF16)
            nc.sync.dma_start(mib_s[:], mib_in[:])

            ident = singles.tile([128, 128], BF16)
            make_identity(nc, ident[:])
            ones = singles.tile([128, 1], BF16)
            nc.vector.memset(ones[:], 1.0)

            z_ps = [zpsum.tile([128, SQ], F32, name=f"z{t}", tag=f"z{t}")
                    for t in range(H // 2)]

            # software-pipelined emission: PE program order interleaves the
            # next block's QK matmuls between this block's head-sum and AV
            # matmuls, so the PE never idles long enough to re-throttle (HAM)
            u_tiles, rb_tiles, att_tiles = {}, {}, {}

            def emit_qk_exp(kb):
                kT_t = ktpool.tile([128, H // 2, 128], F32R, tag="kt",
                                   name=f"kt{kb}")
                nc.sync.dma_start(kT_t[:], kT_in[kb])
                u_all = upool.tile([128, H, SQ], BF16, tag="u_all",
                                   name=f"u{kb}")
                u_tiles[kb] = u_all
                for h in range(H):
                    g, sub = h // 2, h % 2
                    s_ps = spsum.tile([128, SQ], F32, tag="s", name=f"s{kb}_{h}")
                    # heads pair up on row-groups 0/64 of the PE array and
                    # run concurrently (K=64 each)
                    nc.tensor.matmul(
                        s_ps[:],
                        lhsT=kT_t[sub * 64:(sub + 1) * 64, g, :],
                        rhs=qT_s[sub * 64:(sub + 1) * 64, g, :],
                        start=True, stop=True,
                    )
                    nc.scalar.activation(
                        u_all[:, h, :], s_ps[:],
                        mybir.ActivationFunctionType.Exp, scale=ALPHA,
                    )

            def emit_mask_sum(kb):
                u_all = u_tiles[kb]
                # masked positions -> u = 1.0 so the head softmax gives 1/8.
                # Every 3rd block runs as mult+add on the otherwise-idle
                # GPSIMD engine; the rest use DVE copy_predicated.
                if kb % 3 == 2:
                    nc.gpsimd.tensor_mul(
                        u_all[:], u_all[:], _bcast(mf_s[:, kb, :], H))
                    nc.gpsimd.tensor_add(
                        u_all[:], u_all[:], _bcast(mib_s[:, kb, :], H))
                else:
                    nc.vector.copy_predicated(
                        u_all[:],
                        _bcast(mib_s[:, kb, :].bitcast(mybir.dt.uint16), H),
                        bass.AP(tensor=ones.tensor, offset=ones.offset,
                                ap=[ones.ap[0], [0, H], [0, SQ]]),
                    )
                U_ps = upsum.tile([128, SQ], F32, tag="U", name=f"U{kb}")
                for h in range(H):
                    nc.tensor.matmul(
                        U_ps[:], lhsT=ident[:], rhs=u_all[:, h, :],
                        start=(h == 0), stop=(h == H - 1),
                    )
                # r = 1/U as exp(-ln U): two scalar-engine ops, keeping the
                # expensive iterative divide off the vector engine
                lnU = rpool.tile([128, SQ], F32, tag="lnU", name=f"lnU{kb}")
                nc.scalar.activation(
                    lnU[:], U_ps[:], mybir.ActivationFunctionType.Ln)
                rb = rpool.tile([128, SQ], BF16, tag="rb", name=f"rb{kb}")
                nc.scalar.activation(
                    rb[:], lnU[:], mybir.ActivationFunctionType.Exp, scale=-1.0)
                rb_tiles[kb] = rb

            def emit_norm_av(kb):
                u_all, rb = u_tiles.pop(kb), rb_tiles.pop(kb)
                att_all = apool.tile([128, H, SQ], BF16, tag="att_all",
                                     name=f"att{kb}")
                nc.vector.tensor_mul(att_all[:], u_all[:], _bcast(rb[:], H))
                nc.sync.dma_start(att_out[kb], att_all[:])
                for h in range(H):
                    t, bp = h // 2, (h % 2) * 64
                    nc.tensor.matmul(
                        z_ps[t][bp:bp + 64, :],
                        lhsT=v_s[:, h, kb, :],
                        rhs=att_all[:, h, :],
                        start=(kb == 0), stop=(kb == KB - 1),
                        skip_group_check=True,
                    )

            for kb in range(KB + 2):
                if kb < KB:
                    emit_qk_exp(kb)
                if 0 <= kb - 1 < KB:
                    emit_mask_sum(kb - 1)
                if 0 <= kb - 2 < KB:
                    emit_norm_av(kb - 2)
            for t in range(H // 2):
                zsb = zpool.tile([128, SQ], F32, tag="zsb")
                nc.scalar.copy(zsb[:], z_ps[t][:])
                nc.sync.dma_start(z_out[t], zsb[:])

    _split_excess_waits(nc)
    return nc


_PROGRAM = None


def _get_program():
    global _PROGRAM
    if _PROGRAM is None:
        _PROGRAM = _build_program()
    return _PROGRAM


def _make_in_maps(q, k, v, mask):
    import ml_dtypes
    q = np.ascontiguousarray(q, dtype=np.float32)
    k = np.ascontiguousarray(k, dtype=np.float32)
    v = np.ascontiguousarray(v, dtype=np.float32)
    mask = np.asarray(mask)
    in_maps = []
    for c in range(NCORES):
        b, qs = c // (NCORES // B), (c % (NCORES // B)) * SQ
        # qT[d + 64*(h%2), h//2, i] = q[b, h, qs+i, d]  (row-group packing)
        qT = q[b, :, qs:qs + SQ, :].transpose(2, 0, 1)        # [D, H, SQ]
        qT = qT.reshape(D, H // 2, 2, SQ).transpose(2, 0, 1, 3)
        qT = np.ascontiguousarray(qT.reshape(128, H // 2, SQ))
        # kT[kb, d + 64*(h%2), h//2, j] = k[b, h, kb*128+j, d]
        kT = k[b].transpose(2, 0, 1)                          # [D, H, S]
        kT = kT.reshape(D, H // 2, 2, KB, 128).transpose(3, 2, 0, 1, 4)
        kT = np.ascontiguousarray(kT.reshape(KB, 128, H // 2, 128))
        vc = np.ascontiguousarray(
            v[b].reshape(H, KB, 128, D).transpose(2, 0, 1, 3)).astype(
                ml_dtypes.bfloat16)
        mi = (mask[b, qs:qs + SQ, :] == 0).T                  # [S, SQ] bool
        mi = np.ascontiguousarray(
            mi.reshape(KB, 128, SQ).transpose(1, 0, 2))
        mib = mi.astype(ml_dtypes.bfloat16)
        mf = (~mi).astype(ml_dtypes.bfloat16)
        in_maps.append({"qT": qT, "kT": kT, "v": vc, "mf": mf, "mib": mib})
    return in_maps


def _assemble(results):
    z = np.empty((B, H, S, D), dtype=np.float32)
    att = np.empty((B, H, S, S), dtype=np.float32)
    for c in range(NCORES):
        b, qs = c // (NCORES // B), (c % (NCORES // B)) * SQ
        a = np.asarray(results[c]["att"]).astype(np.float32)
        att[b, :, qs:qs + SQ, :] = (
            a.transpose(2, 3, 0, 1).reshape(H, SQ, S))
        zc = results[c]["z"]           # [H//2, 128, SQ]
        z[b, :, qs:qs + SQ, :] = (
            zc.reshape(H, D, SQ).transpose(0, 2, 1))
    return z, att


def _run(q, k, v, mask, trace=False):
    from concourse.bass_utils import run_bass_kernel_spmd

    nc = _get_program()
    in_maps = _make_in_maps(q, k, v, mask)
    res = run_bass_kernel_spmd(nc, in_maps, list(range(NCORES)), trace=trace)
    z, att = _assemble(res.results)
    return z, att, res


def kernel(q, k, v, mask):
    z, att, _ = _run(q, k, v, mask)
    return z, att


# revision 15
# speedup vs baseline: 1.4297x; 1.4297x over previous
"""Bass/Trainium2 kernel for nn_Attention_61194694034287.

Computes, for q,k,v [2,8,2048,64] f32 and mask [2,2048,2048] i32:
    s   = (q @ k^T) / sqrt(8)            # scale by sqrt(n_heads) quirk
    s   = where(mask==0, -1e11, s)
    att = softmax(s, axis=1)             # softmax over the HEAD axis quirk
    z   = att @ v
returns (z, att) like the reference.

Strategy (8 NeuronCores): core c handles batch c//4, query rows
[(c%4)*512, (c%4)*512+512).  All 8 heads live on every core, so the
head-axis softmax is core-local (no collective needed).

Device-side layout is the *transposed* score layout sT[k, q]: the k->q
orientation makes the att @ v contraction a natural PE matmul (k on
partitions for both att^T and v), avoiding any on-device transpose of the
64M-element score tensor.  The host permutes the score slabs back during
unshard (a cheap numpy transpose).

Head-softmax without max-subtraction: score range is ~±17 after scaling,
safely inside fp32 exp.  Masked positions must come out exactly uniform
(1/8) across heads -- reference adds -1e11 which swamps the scores -- so
we set u=exp(s)=1 at masked positions via copy_predicated and the head-sum
is then exactly 8 there.

Head sum runs on the PE as 8 accumulating identity matmuls; normalize
(u * 1/U) on the vector engine; exp on the scalar engine; QK^T and AV on
the PE in float32r (full-rate fp32 path).
"""

import math
import sys
import types

import numpy as np

for _p in ("/opt/trn_rl_repo",):
    if _p not in sys.path:
        sys.path.append(_p)

import concourse.bass as bass
import concourse.tile as tile
from concourse import mybir
from concourse.masks import make_identity
from concourse.vector_clock import ScopedClock

B, H, S, D = 2, 8, 2048, 64
NCORES = 8
SQ = S * B // NCORES          # 512 query rows per core
KB = S // 128                 # 16 k-blocks of 128
ALPHA = 1.0 / math.sqrt(H)    # scale by sqrt(n_heads) (reference quirk)

F32 = mybir.dt.float32
F32R = mybir.dt.float32r
BF16 = mybir.dt.bfloat16
U8 = mybir.dt.uint8

# ---------------------------------------------------------------------------
# Tile's end-of-kernel drain carries one sem-wait per outstanding semaphore;
# the walrus build here rejects >1 sync wait on the SP drain.  Split the
# extras onto single-wait instructions (same engine, before the barrier).
# ---------------------------------------------------------------------------
def _patched_drain_and_barrier(self, tick_clock, wait_clock):
    nc = self.nc
    drain_inst = nc.sync.drain()
    raw = drain_inst.ins
    wait_clock.add_sem_waits(raw, ScopedClock({None: tick_clock.global_clock}))
    si = raw.sync_info
    waits = list(si.on_wait)
    if len(waits) > 1:
        sems_by_name = {h.name: h for h in self.sems.allocated().values()}
        si.on_wait = waits[:1]
        raw.sync_info = si
        for w in waits[1:]:
            nc.sync.wait_ge(sems_by_name[w.ant_name], w.wait_value)
    nc.all_engine_barrier()
    popped = nc._tile_sem_poison_stack.pop()
    assert popped is self._sem_poison
    nc.clear_and_free_semaphores(list(self.sems.allocated().values()))
    nc.all_engine_barrier()


tile.TileContext._drain_and_barrier = _patched_drain_and_barrier

_MAX_WAITS = 1


def _split_excess_waits(nc):
    """walrus rejects instructions with more than ~1 sync wait; hoist the
    extras onto same-engine NOPs inserted right before the instruction."""
    counter = [0]
    for fn in nc.m.functions:
        for bb in fn.blocks:
            new_insts = []
            for inst in bb.instructions:
                si = inst.sync_info
                if si is not None and len(si.on_wait) > _MAX_WAITS:
                    waits = list(si.on_wait)
                    SI = type(si)
                    for w in waits[:-_MAX_WAITS]:
                        counter[0] += 1
                        nop = mybir.InstNoOp(
                            name=f"wsplit-{counter[0]}", ins=[], outs=[])
                        nop.engine = inst.engine
                        nop.sync_info = SI(on_wait=[w], on_update=[])
                        nc.register_instruction(nop, overwrite=True)
                        new_insts.append(nop)
                    si.on_wait = waits[-_MAX_WAITS:]
                    inst.sync_info = si
                new_insts.append(inst)
            bb.instructions = new_insts


def _bcast(ap2d, n, axis=1):
    """[128, F] AP -> [128, n, F] AP with a step-0 (broadcast) middle dim."""
    return bass.AP(
        tensor=ap2d.tensor,
        offset=ap2d.offset,
        ap=[ap2d.ap[0], [0, n], ap2d.ap[1]],
    )


def _build_program():
    nc = bass.Bass("TRN2", target_bir_lowering=False, debug=False,
                   num_devices=NCORES)

    qT_in = nc.dram_tensor("qT", [128, H // 2, SQ], F32R, kind="ExternalInput")
    kT_in = nc.dram_tensor("kT", [KB, 128, H // 2, 128], F32R, kind="ExternalInput")
    v_in = nc.dram_tensor("v", [128, H, KB, D], BF16, kind="ExternalInput")
    mib_in = nc.dram_tensor("mib", [128, KB, SQ], BF16, kind="ExternalInput")
    att_out = nc.dram_tensor("att", [KB, 128, H, SQ], BF16, kind="ExternalOutput")
    z_out = nc.dram_tensor("z", [H // 2, 128, SQ], F32, kind="ExternalOutput")

    with tile.TileContext(nc) as tc:
        with (
            tc.tile_pool(name="singles", bufs=1) as singles,
            tc.tile_pool(name="sb_in", bufs=1) as sb_in,
            tc.tile_pool(name="u", bufs=3) as upool,
            tc.tile_pool(name="kt", bufs=3) as ktpool,
            tc.tile_pool(name="att", bufs=3) as apool,
            tc.tile_pool(name="r", bufs=3) as rpool,
            tc.tile_pool(name="zout", bufs=2) as zpool,
            tc.tile_pool(name="spsum", bufs=3, space="PSUM") as spsum,
            tc.tile_pool(name="upsum", bufs=1, space="PSUM") as upsum,
            tc.tile_pool(name="zpsum", bufs=1, space="PSUM") as zpsum,
        ):
            qT_s = sb_in.tile([128, H // 2, SQ], F32R)
            nc.sync.dma_start(qT_s[:], qT_in[:])
            v_s = sb_in.tile([128, H, KB, D], BF16)
            nc.sync.dma_start(v_s[:], v_in[:])
            mib_s = sb_in.tile([128, KB, SQ], BF16)
            nc.sync.dma_start(mib_s[:], mib_in[:])

            ident = singles.tile([128, 128], BF16)
            make_identity(nc, ident[:])
            ones = singles.tile([128, 1], BF16)
            nc.vector.memset(ones[:], 1.0)

            z_ps = [zpsum.tile([128, SQ], F32, name=f"z{t}", tag=f"z{t}")
                    for t in range(H // 2)]

            # software-pipelined emission: PE program order interleaves the
            # next block's QK matmuls between this block's head-sum and AV
            # matmuls, so the PE never idles long enough to re-throttle (HAM)
            u_tiles, rb_tiles, att_tiles = {}, {}, {}

            def emit_qk_exp(kb):
                kT_t = ktpool.tile([128, H // 2, 128], F32R, tag="kt",
                                   name=f"kt{kb}")
                nc.sync.dma_start(kT_t[:], kT_in[kb])
                u_all = upool.tile([128, H, SQ], BF16, tag="u_all",
                                   name=f"u{kb}")
                u_tiles[kb] = u_all
                for h in range(H):
                    g, sub = h // 2, h % 2
                    s_ps = spsum.tile([128, SQ], F32, tag="s", name=f"s{kb}_{h}")
                    # heads pair up on row-groups 0/64 of the PE array and
                    # run concurrently (K=64 each)
                    nc.tensor.matmul(
                        s_ps[:],
                        lhsT=kT_t[sub * 64:(sub + 1) * 64, g, :],
                        rhs=qT_s[sub * 64:(sub + 1) * 64, g, :],
                        start=True, stop=True,
                    )
                    nc.scalar.activation(
                        u_all[:, h, :], s_ps[:],
                        mybir.ActivationFunctionType.Exp, scale=ALPHA,
                    )

            def emit_mask_sum(kb):
                u_all = u_tiles[kb]
                # masked positions -> u = 1.0 so the head softmax gives 1/8
                nc.vector.copy_predicated(
                    u_all[:],
                    _bcast(mib_s[:, kb, :].bitcast(mybir.dt.uint16), H),
                    bass.AP(tensor=ones.tensor, offset=ones.offset,
                            ap=[ones.ap[0], [0, H], [0, SQ]]),
                )
                U_ps = upsum.tile([128, SQ], F32, tag="U", name=f"U{kb}")
                for h in range(H):
                    nc.tensor.matmul(
                        U_ps[:], lhsT=ident[:], rhs=u_all[:, h, :],
                        start=(h == 0), stop=(h == H - 1),
                    )
                # r = 1/U as exp(-ln U): two scalar-engine ops, keeping the
                # expensive iterative divide off the vector engine
                lnU = rpool.tile([128, SQ], F32, tag="lnU", name=f"lnU{kb}")
                nc.scalar.activation(
                    lnU[:], U_ps[:], mybir.ActivationFunctionType.Ln)
                rb = rpool.tile([128, SQ], BF16, tag="rb", name=f"rb{kb}")
                nc.scalar.activation(
                    rb[:], lnU[:], mybir.ActivationFunctionType.Exp, scale=-1.0)
                rb_tiles[kb] = rb

            def emit_norm_av(kb):
                u_all, rb = u_tiles.pop(kb), rb_tiles.pop(kb)
                att_all = apool.tile([128, H, SQ], BF16, tag="att_all",
                                     name=f"att{kb}")
                nc.vector.tensor_mul(att_all[:], u_all[:], _bcast(rb[:], H))
                nc.sync.dma_start(att_out[kb], att_all[:])
                for h in range(H):
                    t, bp = h // 2, (h % 2) * 64
                    nc.tensor.matmul(
                        z_ps[t][bp:bp + 64, :],
                        lhsT=v_s[:, h, kb, :],
                        rhs=att_all[:, h, :],
                        start=(kb == 0), stop=(kb == KB - 1),
                        skip_group_check=True,
                    )

            for kb in range(KB + 2):
                if kb < KB:
                    emit_qk_exp(kb)
                if 0 <= kb - 1 < KB:
                    emit_mask_sum(kb - 1)
                if 0 <= kb - 2 < KB:
                    emit_norm_av(kb - 2)
            for t in range(H // 2):
                zsb = zpool.tile([128, SQ], F32, tag="zsb")
                nc.scalar.copy(zsb[:], z_ps[t][:])
                nc.sync.dma_start(z_out[t], zsb[:])

    _split_excess_waits(nc)
    return nc


_PROGRAM = None


def _get_program():
    global _PROGRAM
    if _PROGRAM is None:
        _PROGRAM = _build_program()
    return _PROGRAM


def _make_in_maps(q, k, v, mask):
    import ml_dtypes
    q = np.ascontiguousarray(q, dtype=np.float32)
    k = np.ascontiguousarray(k, dtype=np.float32)
    v = np.ascontiguousarray(v, dtype=np.float32)
    mask = np.asarray(mask)
    in_maps = []
    for c in range(NCORES):
        b, qs = c // (NCORES // B), (c % (NCORES // B)) * SQ
        # qT[d + 64*(h%2), h//2, i] = q[b, h, qs+i, d]  (row-group packing)
        qT = q[b, :, qs:qs + SQ, :].transpose(2, 0, 1)        # [D, H, SQ]
        qT = qT.reshape(D, H // 2, 2, SQ).transpose(2, 0, 1, 3)
        qT = np.ascontiguousarray(qT.reshape(128, H // 2, SQ))
        # kT[kb, d + 64*(h%2), h//2, j] = k[b, h, kb*128+j, d]
        kT = k[b].transpose(2, 0, 1)                          # [D, H, S]
        kT = kT.reshape(D, H // 2, 2, KB, 128).transpose(3, 2, 0, 1, 4)
        kT = np.ascontiguousarray(kT.reshape(KB, 128, H // 2, 128))
        vc = np.ascontiguousarray(
            v[b].reshape(H, KB, 128, D).transpose(2, 0, 1, 3)).astype(
                ml_dtypes.bfloat16)
        mi = (mask[b, qs:qs + SQ, :] == 0).T                  # [S, SQ] bool
        mi = np.ascontiguousarray(
            mi.reshape(KB, 128, SQ).transpose(1, 0, 2))
        mib = mi.astype(ml_dtypes.bfloat16)
        in_maps.append({"qT": qT, "kT": kT, "v": vc, "mib": mib})
    return in_maps


def _assemble(results):
    z = np.empty((B, H, S, D), dtype=np.float32)
    att = np.empty((B, H, S, S), dtype=np.float32)
    for c in range(NCORES):
        b, qs = c // (NCORES // B), (c % (NCORES // B)) * SQ
        a = np.asarray(results[c]["att"]).astype(np.float32)
        att[b, :, qs:qs + SQ, :] = (
            a.transpose(2, 3, 0, 1).reshape(H, SQ, S))
        zc = results[c]["z"]           # [H//2, 128, SQ]
        z[b, :, qs:qs + SQ, :] = (
            zc.reshape(H, D, SQ).transpose(0, 2, 1))
    return z, att


def _run(q, k, v, mask, trace=False):
    from concourse.bass_utils import run_bass_kernel_spmd

    nc = _get_program()
    in_maps = _make_in_maps(q, k, v, mask)
    res = run_bass_kernel_spmd(nc, in_maps, list(range(NCORES)), trace=trace)
    z, att = _assemble(res.results)
    return z, att, res


def kernel(q, k, v, mask):
    z, att, _ = _run(q, k, v, mask)
    return z, att


# revision 16
# speedup vs baseline: 1.5001x; 1.0493x over previous
"""Bass/Trainium2 kernel for nn_Attention_61194694034287.

Computes, for q,k,v [2,8,2048,64] f32 and mask [2,2048,2048] i32:
    s   = (q @ k^T) / sqrt(8)            # scale by sqrt(n_heads) quirk
    s   = where(mask==0, -1e11, s)
    att = softmax(s, axis=1)             # softmax over the HEAD axis quirk
    z   = att @ v
returns (z, att) like the reference.

Strategy (8 NeuronCores): core c handles batch c//4, query rows
[(c%4)*512, (c%4)*512+512).  All 8 heads live on every core, so the
head-axis softmax is core-local (no collective needed).

Device-side layout is the *transposed* score layout sT[k, q]: the k->q
orientation makes the att @ v contraction a natural PE matmul (k on
partitions for both att^T and v), avoiding any on-device transpose of the
64M-element score tensor.  The host permutes the score slabs back during
unshard (a cheap numpy transpose).

Head-softmax without max-subtraction: score range is ~±17 after scaling,
safely inside fp32 exp.  Masked positions must come out exactly uniform
(1/8) across heads -- reference adds -1e11 which swamps the scores -- so
we set u=exp(s)=1 at masked positions via copy_predicated and the head-sum
is then exactly 8 there.

Head sum runs on the PE as 8 accumulating identity matmuls; normalize
(u * 1/U) on the vector engine; exp on the scalar engine; QK^T and AV on
the PE in float32r (full-rate fp32 path).
"""

import math
import sys
import types

import numpy as np

for _p in ("/opt/trn_rl_repo",):
    if _p not in sys.path:
        sys.path.append(_p)

import concourse.bass as bass
import concourse.tile as tile
from concourse import mybir
from concourse.masks import make_identity
from concourse.vector_clock import ScopedClock

B, H, S, D = 2, 8, 2048, 64
NCORES = 8
SQ = S * B // NCORES          # 512 query rows per core
KB = S // 128                 # 16 k-blocks of 128
ALPHA = 1.0 / math.sqrt(H)    # scale by sqrt(n_heads) (reference quirk)

F32 = mybir.dt.float32
F32R = mybir.dt.float32r
BF16 = mybir.dt.bfloat16
U8 = mybir.dt.uint8

# ---------------------------------------------------------------------------
# Tile's end-of-kernel drain carries one sem-wait per outstanding semaphore;
# the walrus build here rejects >1 sync wait on the SP drain.  Split the
# extras onto single-wait instructions (same engine, before the barrier).
# ---------------------------------------------------------------------------
def _patched_drain_and_barrier(self, tick_clock, wait_clock):
    nc = self.nc
    drain_inst = nc.sync.drain()
    raw = drain_inst.ins
    wait_clock.add_sem_waits(raw, ScopedClock({None: tick_clock.global_clock}))
    si = raw.sync_info
    waits = list(si.on_wait)
    if len(waits) > 1:
        sems_by_name = {h.name: h for h in self.sems.allocated().values()}
        si.on_wait = waits[:1]
        raw.sync_info = si
        for w in waits[1:]:
            nc.sync.wait_ge(sems_by_name[w.ant_name], w.wait_value)
    nc.all_engine_barrier()
    popped = nc._tile_sem_poison_stack.pop()
    assert popped is self._sem_poison
    nc.clear_and_free_semaphores(list(self.sems.allocated().values()))
    nc.all_engine_barrier()


tile.TileContext._drain_and_barrier = _patched_drain_and_barrier

_MAX_WAITS = 1


def _split_excess_waits(nc):
    """walrus rejects instructions with more than ~1 sync wait; hoist the
    extras onto same-engine NOPs inserted right before the instruction."""
    counter = [0]
    for fn in nc.m.functions:
        for bb in fn.blocks:
            new_insts = []
            for inst in bb.instructions:
                si = inst.sync_info
                if si is not None and len(si.on_wait) > _MAX_WAITS:
                    waits = list(si.on_wait)
                    SI = type(si)
                    for w in waits[:-_MAX_WAITS]:
                        counter[0] += 1
                        nop = mybir.InstNoOp(
                            name=f"wsplit-{counter[0]}", ins=[], outs=[])
                        nop.engine = inst.engine
                        nop.sync_info = SI(on_wait=[w], on_update=[])
                        nc.register_instruction(nop, overwrite=True)
                        new_insts.append(nop)
                    si.on_wait = waits[-_MAX_WAITS:]
                    inst.sync_info = si
                new_insts.append(inst)
            bb.instructions = new_insts


def _bcast(ap2d, n, axis=1):
    """[128, F] AP -> [128, n, F] AP with a step-0 (broadcast) middle dim."""
    return bass.AP(
        tensor=ap2d.tensor,
        offset=ap2d.offset,
        ap=[ap2d.ap[0], [0, n], ap2d.ap[1]],
    )


def _build_program():
    nc = bass.Bass("TRN2", target_bir_lowering=False, debug=False,
                   num_devices=NCORES)

    qT_in = nc.dram_tensor("qT", [128, H // 2, SQ], F32R, kind="ExternalInput")
    kT_in = nc.dram_tensor("kT", [KB, 128, H // 2, 128], F32R, kind="ExternalInput")
    v_in = nc.dram_tensor("v", [KB, 128, H, D], BF16, kind="ExternalInput")
    mib_in = nc.dram_tensor("mib", [KB, 128, SQ], BF16, kind="ExternalInput")
    att_out = nc.dram_tensor("att", [KB, 128, H, SQ], BF16, kind="ExternalOutput")
    z_out = nc.dram_tensor("z", [H // 2, 128, SQ], F32, kind="ExternalOutput")

    with tile.TileContext(nc) as tc:
        with (
            tc.tile_pool(name="singles", bufs=1) as singles,
            tc.tile_pool(name="sb_in", bufs=1) as sb_in,
            tc.tile_pool(name="u", bufs=3) as upool,
            tc.tile_pool(name="kt", bufs=3) as ktpool,
            tc.tile_pool(name="mib", bufs=3) as mibpool,
            tc.tile_pool(name="v", bufs=4) as vpool,
            tc.tile_pool(name="att", bufs=3) as apool,
            tc.tile_pool(name="r", bufs=3) as rpool,
            tc.tile_pool(name="zout", bufs=2) as zpool,
            tc.tile_pool(name="spsum", bufs=3, space="PSUM") as spsum,
            tc.tile_pool(name="upsum", bufs=1, space="PSUM") as upsum,
            tc.tile_pool(name="zpsum", bufs=1, space="PSUM") as zpsum,
        ):
            qT_s = sb_in.tile([128, H // 2, SQ], F32R)
            nc.sync.dma_start(qT_s[:], qT_in[:])

            ident = singles.tile([128, 128], BF16)
            make_identity(nc, ident[:])
            ones = singles.tile([128, 1], BF16)
            nc.vector.memset(ones[:], 1.0)

            z_ps = [zpsum.tile([128, SQ], F32, name=f"z{t}", tag=f"z{t}")
                    for t in range(H // 2)]

            # software-pipelined emission: PE program order interleaves the
            # next block's QK matmuls between this block's head-sum and AV
            # matmuls, so the PE never idles long enough to re-throttle (HAM)
            u_tiles, rb_tiles, att_tiles = {}, {}, {}

            v_tiles, mib_tiles = {}, {}

            def emit_qk_exp(kb):
                kT_t = ktpool.tile([128, H // 2, 128], F32R, tag="kt",
                                   name=f"kt{kb}")
                nc.sync.dma_start(kT_t[:], kT_in[kb])
                mib_t = mibpool.tile([128, SQ], BF16, tag="mib",
                                     name=f"mib{kb}")
                nc.sync.dma_start(mib_t[:], mib_in[kb])
                mib_tiles[kb] = mib_t
                v_t = vpool.tile([128, H, D], BF16, tag="v", name=f"v{kb}")
                nc.sync.dma_start(v_t[:], v_in[kb])
                v_tiles[kb] = v_t
                u_all = upool.tile([128, H, SQ], BF16, tag="u_all",
                                   name=f"u{kb}")
                u_tiles[kb] = u_all
                for h in range(H):
                    g, sub = h // 2, h % 2
                    s_ps = spsum.tile([128, SQ], F32, tag="s", name=f"s{kb}_{h}")
                    # heads pair up on row-groups 0/64 of the PE array and
                    # run concurrently (K=64 each)
                    nc.tensor.matmul(
                        s_ps[:],
                        lhsT=kT_t[sub * 64:(sub + 1) * 64, g, :],
                        rhs=qT_s[sub * 64:(sub + 1) * 64, g, :],
                        start=True, stop=True,
                    )
                    nc.scalar.activation(
                        u_all[:, h, :], s_ps[:],
                        mybir.ActivationFunctionType.Exp, scale=ALPHA,
                    )

            def emit_mask_sum(kb):
                u_all = u_tiles[kb]
                # masked positions -> u = 1.0 so the head softmax gives 1/8
                mib_t = mib_tiles[kb]
                nc.vector.copy_predicated(
                    u_all[:],
                    _bcast(mib_t[:].bitcast(mybir.dt.uint16), H),
                    bass.AP(tensor=ones.tensor, offset=ones.offset,
                            ap=[ones.ap[0], [0, H], [0, SQ]]),
                )
                U_ps = upsum.tile([128, SQ], F32, tag="U", name=f"U{kb}")
                for h in range(H):
                    nc.tensor.matmul(
                        U_ps[:], lhsT=ident[:], rhs=u_all[:, h, :],
                        start=(h == 0), stop=(h == H - 1),
                    )
                # r = 1/U as exp(-ln U): two scalar-engine ops, keeping the
                # expensive iterative divide off the vector engine
                lnU = rpool.tile([128, SQ], F32, tag="lnU", name=f"lnU{kb}")
                nc.scalar.activation(
                    lnU[:], U_ps[:], mybir.ActivationFunctionType.Ln)
                rb = rpool.tile([128, SQ], BF16, tag="rb", name=f"rb{kb}")
                nc.scalar.activation(
                    rb[:], lnU[:], mybir.ActivationFunctionType.Exp, scale=-1.0)
                rb_tiles[kb] = rb

            def emit_norm_av(kb):
                u_all, rb = u_tiles.pop(kb), rb_tiles.pop(kb)
                att_all = apool.tile([128, H, SQ], BF16, tag="att_all",
                                     name=f"att{kb}")
                nc.vector.tensor_mul(att_all[:], u_all[:], _bcast(rb[:], H))
                nc.sync.dma_start(att_out[kb], att_all[:])
                v_t = v_tiles.pop(kb)
                for h in range(H):
                    t, bp = h // 2, (h % 2) * 64
                    nc.tensor.matmul(
                        z_ps[t][bp:bp + 64, :],
                        lhsT=v_t[:, h, :],
                        rhs=att_all[:, h, :],
                        start=(kb == 0), stop=(kb == KB - 1),
                        skip_group_check=True,
                    )

            for kb in range(KB + 2):
                if kb < KB:
                    emit_qk_exp(kb)
                if 0 <= kb - 1 < KB:
                    emit_mask_sum(kb - 1)
                if 0 <= kb - 2 < KB:
                    emit_norm_av(kb - 2)
            for t in range(H // 2):
                zsb = zpool.tile([128, SQ], F32, tag="zsb")
                nc.scalar.copy(zsb[:], z_ps[t][:])
                nc.sync.dma_start(z_out[t], zsb[:])

    _split_excess_waits(nc)
    return nc


_PROGRAM = None


def _get_program():
    global _PROGRAM
    if _PROGRAM is None:
        _PROGRAM = _build_program()
    return _PROGRAM


def _make_in_maps(q, k, v, mask):
    import ml_dtypes
    q = np.ascontiguousarray(q, dtype=np.float32)
    k = np.ascontiguousarray(k, dtype=np.float32)
    v = np.ascontiguousarray(v, dtype=np.float32)
    mask = np.asarray(mask)
    in_maps = []
    for c in range(NCORES):
        b, qs = c // (NCORES // B), (c % (NCORES // B)) * SQ
        # qT[d + 64*(h%2), h//2, i] = q[b, h, qs+i, d]  (row-group packing)
        qT = q[b, :, qs:qs + SQ, :].transpose(2, 0, 1)        # [D, H, SQ]
        qT = qT.reshape(D, H // 2, 2, SQ).transpose(2, 0, 1, 3)
        qT = np.ascontiguousarray(qT.reshape(128, H // 2, SQ))
        # kT[kb, d + 64*(h%2), h//2, j] = k[b, h, kb*128+j, d]
        kT = k[b].transpose(2, 0, 1)                          # [D, H, S]
        kT = kT.reshape(D, H // 2, 2, KB, 128).transpose(3, 2, 0, 1, 4)
        kT = np.ascontiguousarray(kT.reshape(KB, 128, H // 2, 128))
        vc = np.ascontiguousarray(
            v[b].reshape(H, KB, 128, D).transpose(1, 2, 0, 3)).astype(
                ml_dtypes.bfloat16)
        mi = (mask[b, qs:qs + SQ, :] == 0).T                  # [S, SQ] bool
        mib = np.ascontiguousarray(
            mi.reshape(KB, 128, SQ)).astype(ml_dtypes.bfloat16)
        in_maps.append({"qT": qT, "kT": kT, "v": vc, "mib": mib})
    return in_maps


def _assemble(results):
    z = np.empty((B, H, S, D), dtype=np.float32)
    att = np.empty((B, H, S, S), dtype=np.float32)
    for c in range(NCORES):
        b, qs = c // (NCORES // B), (c % (NCORES // B)) * SQ
        a = np.asarray(results[c]["att"]).astype(np.float32)
        att[b, :, qs:qs + SQ, :] = (
            a.transpose(2, 3, 0, 1).reshape(H, SQ, S))
        zc = results[c]["z"]           # [H//2, 128, SQ]
        z[b, :, qs:qs + SQ, :] = (
            zc.reshape(H, D, SQ).transpose(0, 2, 1))
    return z, att


def _run(q, k, v, mask, trace=False):
    from concourse.bass_utils import run_bass_kernel_spmd

    nc = _get_program()
    in_maps = _make_in_maps(q, k, v, mask)
    res = run_bass_kernel_spmd(nc, in_maps, list(range(NCORES)), trace=trace)
    z, att = _assemble(res.results)
    return z, att, res


def kernel(q, k, v, mask):
    z, att, _ = _run(q, k, v, mask)
    return z, att
